# revision 1
# baseline (speedup 1.0000x reference)
"""Trainium2 Bass kernel for nn_MDRMWithCPRecon.

Sharding: pure data parallel over batch B=8 -> one batch element per
NeuronCore (8 cores). All parameters replicated. Each core computes the
full per-batch pipeline:

  x = cat(frm, oth)                 [512, 64, 64]
  Fm = lrelu(conv3x3(x, W3) + b3)   [256, 64, 64]   <- bulk of FLOPs
  U1/U2/U3 rank-4 softmax factors from pooled stats (tiny matmuls)
  spatial  = sigmoid(ws * U3 @ U2^T + bs)
  spectral = sigmoid(sigmoid(Wsa@mean + Wsm@max + biases))
  Wt = spectral x spatial
  fused    = a*Wt*frm + (1-a)*(1-Wt)*oth
  cp_recon = (Wr @ cp + br) * Wt + Fm,  cp = rank-4 CP(U1,U2,U3,lam)

Key tricks:
  - conv3x3 as 9-tap PSUM-accumulated matmuls on a zero-padded [66,66]
    image, channels on partitions, float32r inputs (full-rate matmul at
    ~1e-4 relative accuracy).
  - recon = (Wr @ U1 diag(lam)) @ G with G[r, hw] = U2[h,r]*U3[w,r]:
    turns the [C,C]x[C,HW] recon matmul into a K=4 matmul.
  - spatial map broadcast across partitions via K=4 ones-matmul of G.
  - all weight transposes/layouts are prepared host-side in numpy.
"""

import numpy as np

import concourse.bacc as bacc
import concourse.bass as bass
import concourse.tile as tile
from concourse import mybir, bass_utils

F32 = mybir.dt.float32
F32R = mybir.dt.float32r
BF16 = mybir.dt.bfloat16
AF = mybir.ActivationFunctionType
ALU = mybir.AluOpType
AX = mybir.AxisListType

B, C, H, W, K = 8, 256, 64, 64, 4
HW = H * W
NCORES = 8


def build_program(alpha, ws, bs):
    from concourse.masks import make_identity

    nc = bacc.Bacc("TRN2", target_bir_lowering=False, debug=False,
                   num_devices=NCORES)

    # ---- DRAM I/O (per core) ----
    frm = nc.dram_tensor("frm", [C, H, W], F32, kind="ExternalInput")
    oth = nc.dram_tensor("oth", [C, H, W], F32, kind="ExternalInput")
    w3t_d = nc.dram_tensor("w3t", [128, 4, 9, 256], F32, kind="ExternalInput")
    b3_d = nc.dram_tensor("b3", [128, 2], F32, kind="ExternalInput")
    wa_d = nc.dram_tensor("wa", [1, 2, 3, 256], F32, kind="ExternalInput")
    ba_d = nc.dram_tensor("ba", [128, 3, 2], F32, kind="ExternalInput")
    wu_d = nc.dram_tensor("wu", [128, 2, 4], F32, kind="ExternalInput")
    bu_d = nc.dram_tensor("bu", [4, 1], F32, kind="ExternalInput")
    wrt_d = nc.dram_tensor("wrt", [128, 2, 2, 128], F32, kind="ExternalInput")
    br_d = nc.dram_tensor("br", [128, 2], F32, kind="ExternalInput")
    wsc_d = nc.dram_tensor("wsc", [128, 4, 2, 128], F32, kind="ExternalInput")
    bsc_d = nc.dram_tensor("bsc", [128, 2], F32, kind="ExternalInput")
    lam_d = nc.dram_tensor("lam", [4, 1], F32, kind="ExternalInput")
    fused_o = nc.dram_tensor("fused", [C, H, W], F32, kind="ExternalOutput")
    cpr_o = nc.dram_tensor("cpr", [C, H, W], F32, kind="ExternalOutput")
    fm_scr = nc.dram_tensor("fm_scratch", [2, 8, 128, 512], F32)  # [ct, pt, p, n]

    with tile.TileContext(nc) as tc:
        _build_tile(tc, nc, make_identity, locals(), alpha, ws, bs)
    nc.compile()
    return nc


def _build_tile(tc, nc, make_identity, T, alpha, ws, bs):
    frm, oth = T["frm"], T["oth"]
    w3t_d, b3_d, wa_d, ba_d = T["w3t_d"], T["b3_d"], T["wa_d"], T["ba_d"]
    wu_d, bu_d, wrt_d, br_d = T["wu_d"], T["bu_d"], T["wrt_d"], T["br_d"]
    wsc_d, bsc_d, lam_d = T["wsc_d"], T["bsc_d"], T["lam_d"]
    fused_o, cpr_o, fm_scr = T["fused_o"], T["cpr_o"], T["fm_scr"]

    import contextlib
    ctx = contextlib.ExitStack()
    consts = ctx.enter_context(tc.tile_pool(name="consts", bufs=1))
    stage = ctx.enter_context(tc.tile_pool(name="stage", bufs=2))
    fmring = ctx.enter_context(tc.tile_pool(name="fmring", bufs=3))
    ew = ctx.enter_context(tc.tile_pool(name="ew", bufs=2))
    outr = ctx.enter_context(tc.tile_pool(name="outr", bufs=2))
    ps_conv = ctx.enter_context(tc.tile_pool(name="ps_conv", bufs=2, space="PSUM"))
    ps_spat = ctx.enter_context(tc.tile_pool(name="ps_spat", bufs=2, space="PSUM"))
    ps_rec = ctx.enter_context(tc.tile_pool(name="ps_rec", bufs=2, space="PSUM"))
    ps_sm = ctx.enter_context(tc.tile_pool(name="ps_sm", bufs=2, space="PSUM"))

    # ---- small weights to SBUF ----
    b3_sb = consts.tile([128, 2], F32)
    nc.sync.dma_start(b3_sb[:], b3_d[:])
    wa_sb = consts.tile([1, 2, 3, 256], F32)
    nc.sync.dma_start(wa_sb[:], wa_d[:])
    ba_sb = consts.tile([128, 3, 2], F32)
    nc.sync.dma_start(ba_sb[:], ba_d[:])
    wu_sb = consts.tile([128, 2, 4], F32)
    nc.sync.dma_start(wu_sb[:], wu_d[:])
    bu_sb = consts.tile([4, 1], F32)
    nc.sync.dma_start(bu_sb[:], bu_d[:])
    wrt_sb = consts.tile([128, 2, 2, 128], F32)
    nc.sync.dma_start(wrt_sb[:], wrt_d[:])
    br_sb = consts.tile([128, 2], F32)
    nc.sync.dma_start(br_sb[:], br_d[:])
    wsc_sb = consts.tile([128, 4, 2, 128], F32)
    nc.sync.dma_start(wsc_sb[:], wsc_d[:])
    bsc_sb = consts.tile([128, 2], F32)
    nc.sync.dma_start(bsc_sb[:], bsc_d[:])
    lam_sb = consts.tile([4, 1], F32)
    nc.sync.dma_start(lam_sb[:], lam_d[:])

    ident = consts.tile([128, 128], F32)
    make_identity(nc, ident[:])
    ones128 = consts.tile([128, 1], F32)
    nc.gpsimd.memset(ones128[:], 1.0)
    ones4 = consts.tile([4, 128], BF16)
    nc.gpsimd.memset(ones4[:], 1.0)

    # ---- conv weights: DMA f32 chunks -> round to f32r ----
    w3t_r = consts.tile([128, 4, 9, 256], F32R)
    for kt in range(4):
        stg = stage.tile([128, 2304], F32, tag="stg")
        nc.sync.dma_start(stg[:], w3t_d[:, kt].rearrange("p a b -> p (a b)"))
        nc.vector.tensor_copy(
            w3t_r[:, kt].rearrange("p a b -> p (a b)"), stg[:])

    # ---- padded input image, f32r [128, 4(kt), 66, 66] ----
    # kt 0/1 = frm channels 0:128/128:256; kt 2/3 = oth channels.
    xr = consts.tile([128, 4, 66, 66], F32R)
    xrf = xr.bitcast(F32)
    for src_i, src in enumerate((frm, oth)):
        src_v = src.rearrange("(k p) h w -> p k h w", p=128)
        for kt2 in range(2):
            kt = src_i * 2 + kt2
            for r in range(2):  # two 33-row chunks of the padded image
                stg = stage.tile([128, 33, 66], F32, tag="stg")
                nc.gpsimd.memset(stg[:, :, 0:1], 0.0)
                nc.gpsimd.memset(stg[:, :, 65:66], 0.0)
                if r == 0:
                    nc.gpsimd.memset(stg[:, 0:1, :], 0.0)
                    nc.sync.dma_start(stg[:, 1:33, 1:65],
                                      src_v[:, kt2, 0:32, :])
                else:
                    nc.gpsimd.memset(stg[:, 32:33, :], 0.0)
                    nc.sync.dma_start(stg[:, 0:32, 1:65],
                                      src_v[:, kt2, 32:64, :])
                nc.vector.tensor_copy(xr[:, kt, r * 33:(r + 1) * 33, :],
                                      stg[:])

    # ---- pooled-stat partials (disjoint per-(ct,pt) slices, no in-place) ----
    sums1 = consts.tile([128, 2, 8], F32)       # per-(ct,pt) sums of Fm
    m1_part = consts.tile([128, 2, 8], F32)     # per-(ct,pt) max of Fm
    pp_sum = consts.tile([128, 2, 8, 64], F32)  # per-pt sum over h -> [c, w]
    pp_max = consts.tile([128, 2, 8, 64], F32)
    q_sum = consts.tile([128, 2, 64], F32)      # sum over w  -> [c, h]
    q_max = consts.tile([128, 2, 64], F32)

    # ---- conv3x3 + leaky relu + streaming stats ----
    for pt in range(8):
        for ct in range(2):
            ps = ps_conv.tile([128, 512], F32, tag="conv")
            idx = 0
            for kt in range(4):
                for t in range(9):
                    dy, dx = t // 3, t % 3
                    nc.tensor.matmul(
                        ps[:],
                        w3t_r[:, kt, t, ct * 128:(ct + 1) * 128],
                        xr[:, kt, pt * 8 + dy: pt * 8 + dy + 8, dx: dx + 64],
                        start=(idx == 0), stop=(idx == 35))
                    idx += 1
            fm_t = fmring.tile([128, 512], F32, tag="fmt")
            nc.scalar.activation(fm_t[:], ps[:], AF.Lrelu,
                                 bias=b3_sb[:, ct:ct + 1], alpha=0.01,
                                 accum_out=sums1[:, ct, pt:pt + 1])
            nc.sync.dma_start(fm_scr[ct, pt], fm_t[:])
            blk = fm_t[:].rearrange("p (h w) -> p h w", h=8)
            blk_t = fm_t[:].rearrange("p (h w) -> p w h", h=8)
            # Q (sum/max over w): disjoint h-slices, write directly
            nc.vector.tensor_reduce(q_sum[:, ct, pt * 8:(pt + 1) * 8], blk,
                                    axis=AX.X, op=ALU.add)
            nc.vector.tensor_reduce(q_max[:, ct, pt * 8:(pt + 1) * 8], blk,
                                    axis=AX.X, op=ALU.max)
            # P (sum/max over h): per-pt partials, combined after the loop
            nc.vector.tensor_reduce(pp_sum[:, ct, pt, :], blk_t,
                                    axis=AX.X, op=ALU.add)
            nc.vector.tensor_reduce(pp_max[:, ct, pt, :], blk_t,
                                    axis=AX.X, op=ALU.max)
            nc.vector.tensor_reduce(m1_part[:, ct, pt:pt + 1], blk,
                                    axis=AX.XY, op=ALU.max)

    # ---- combine partials ----
    max1 = consts.tile([128, 2], F32)
    nc.vector.tensor_reduce(max1[:], m1_part[:], axis=AX.X, op=ALU.max)
    p_sum = consts.tile([128, 2, 64], F32)
    nc.vector.tensor_reduce(
        p_sum[:], pp_sum[:].rearrange("p c t w -> p c w t"),
        axis=AX.X, op=ALU.add)
    p_max = consts.tile([128, 2, 64], F32)
    nc.vector.tensor_reduce(
        p_max[:], pp_max[:].rearrange("p c t w -> p c w t"),
        axis=AX.X, op=ALU.max)

    # ---- mode pooled rows: avg_row_m / max_row_m as [1, N] tiles ----
    sum1 = consts.tile([128, 2], F32)
    nc.vector.tensor_reduce(sum1[:], sums1[:], axis=AX.X, op=ALU.add)
    avg_row1 = consts.tile([1, 256], F32)
    max_row1 = consts.tile([1, 256], F32)
    for ct in range(2):
        tp = ps_sm.tile([1, 128], F32, tag="sm")
        nc.tensor.transpose(tp[:], sum1[:, ct:ct + 1], ident[:])
        nc.scalar.mul(avg_row1[0:1, ct * 128:(ct + 1) * 128], tp[:], 1.0 / HW)
        tp2 = ps_sm.tile([1, 128], F32, tag="sm")
        nc.tensor.transpose(tp2[:], max1[:, ct:ct + 1], ident[:])
        nc.scalar.copy(max_row1[0:1, ct * 128:(ct + 1) * 128], tp2[:])

    def colstats(S_sum, S_max, denom, uid):
        """[128, 2, 64] stats -> ([1,64] avg row, [1,64] max row)."""
        avg_row = consts.tile([1, 64], F32, tag=f"avgr{uid}")
        max_row = consts.tile([1, 64], F32, tag=f"maxr{uid}")
        ssum = ps_sm.tile([1, 64], F32, tag="sm")
        nc.tensor.matmul(ssum[:], ones128[:], S_sum[:, 0, :], start=True,
                         stop=False)
        nc.tensor.matmul(ssum[:], ones128[:], S_sum[:, 1, :], start=False,
                         stop=True)
        nc.scalar.mul(avg_row[:], ssum[:], 1.0 / denom)
        mx = ew.tile([128, 64], F32, tag="mx")
        nc.vector.tensor_tensor(mx[:], S_max[:, 0, :], S_max[:, 1, :],
                                op=ALU.max)
        mxt = ps_sm.tile([64, 128], F32, tag="sm")
        nc.tensor.transpose(mxt[:], mx[:], ident[:])
        mxr = consts.tile([64, 1], F32, tag=f"mxr{uid}")
        nc.vector.tensor_reduce(mxr[:], mxt[:], axis=AX.X, op=ALU.max)
        mxp = ps_sm.tile([1, 64], F32, tag="sm")
        nc.tensor.transpose(mxp[:], mxr[:], ident[0:64, 0:64])
        nc.scalar.copy(max_row[:], mxp[:])
        return avg_row, max_row

    avg_row2, max_row2 = colstats(p_sum, p_max, C * H, "m2")
    avg_row3, max_row3 = colstats(q_sum, q_max, C * W, "m3")

    # ---- U factors ----
    def make_U(m, avg_row, max_row, N):
        """returns Un ([128, nchunk, 4] softmaxed columns) and UT ([4, N])."""
        nch = N // 128 if N >= 128 else 1
        cw = min(N, 128)
        a_sb = consts.tile([128, 2, N], F32, tag=f"a{m}")
        for ct in range(2):
            a_ps = ps_sm.tile([128, N], F32, tag="sm")
            nc.tensor.matmul(a_ps[:], wa_sb[0:1, 0, m, ct * 128:(ct + 1) * 128],
                             avg_row[:], start=True, stop=False)
            nc.tensor.matmul(a_ps[:], wa_sb[0:1, 1, m, ct * 128:(ct + 1) * 128],
                             max_row[:], start=False, stop=True)
            nc.scalar.activation(a_sb[:, ct, :], a_ps[:], AF.Identity,
                                 bias=ba_sb[:, m, ct:ct + 1])
        u_ps = ps_sm.tile([4, N], F32, tag="sm")
        nc.tensor.matmul(u_ps[:], wu_sb[:, 0, :], a_sb[:, 0, :], start=True,
                         stop=False)
        nc.tensor.matmul(u_ps[:], wu_sb[:, 1, :], a_sb[:, 1, :], start=False,
                         stop=True)
        u_sb = consts.tile([4, N], F32, tag=f"u{m}")
        nc.scalar.activation(u_sb[:], u_ps[:], AF.Identity, bias=bu_sb[:])
        Un = consts.tile([cw, nch, 4], F32, tag=f"Un{m}")
        UT = consts.tile([4, N], F32, tag=f"UT{m}")
        for ch in range(nch):
            ut_ps = ps_sm.tile([cw, 4], F32, tag="sm")
            nc.tensor.transpose(ut_ps[:], u_sb[0:4, ch * cw:(ch + 1) * cw],
                                ident[0:4, 0:4])
            negm = ew.tile([cw, 1], F32, tag="negm")
            nc.vector.tensor_reduce(negm[:], ut_ps[:], axis=AX.X, op=ALU.max,
                                    negate=True)
            ssum = ew.tile([cw, 1], F32, tag="ssum")
            etile = ew.tile([cw, 4], F32, tag="etile")
            nc.scalar.activation(etile[:], ut_ps[:], AF.Exp, bias=negm[:],
                                 accum_out=ssum[:])
            rec = ew.tile([cw, 1], F32, tag="rec")
            nc.vector.reciprocal(rec[:], ssum[:])
            nc.vector.tensor_scalar(Un[:, ch, :], etile[:], rec[:], None,
                                    op0=ALU.mult)
            tb_ps = ps_sm.tile([4, cw], F32, tag="sm")
            nc.tensor.transpose(tb_ps[:], Un[:, ch, :], ident[0:cw, 0:cw])
            nc.vector.tensor_copy(UT[:, ch * cw:(ch + 1) * cw], tb_ps[:])
        return Un, UT

    U1n, U1T = make_U(0, avg_row1, max_row1, 256)
    _, U2T = make_U(1, avg_row2, max_row2, 64)
    _, U3T = make_U(2, avg_row3, max_row3, 64)

    # ---- spectral attention ----
    gag = consts.tile([128, 4], F32)  # [ga_ct0, ga_ct1, gm_ct0, gm_ct1]
    for ct in range(2):
        f_ps = ps_sm.tile([128, 128], F32, tag="sm")
        nc.tensor.matmul(f_ps[:, 0:64], U1T[:, ct * 128:(ct + 1) * 128],
                         U2T[:], start=True, stop=True)
        nc.tensor.matmul(f_ps[:, 64:128], U1T[:, ct * 128:(ct + 1) * 128],
                         U3T[:], start=True, stop=True)
        nc.vector.tensor_reduce(gag[:, ct:ct + 1], f_ps[:], axis=AX.X,
                                op=ALU.add)
        nc.vector.tensor_reduce(gag[:, 2 + ct:3 + ct], f_ps[:], axis=AX.X,
                                op=ALU.max)
    spectral = consts.tile([128, 2], F32)
    for mm in range(2):
        sp_ps = ps_sm.tile([128, 1], F32, tag="sm")
        for kk in range(4):
            nc.tensor.matmul(sp_ps[:], wsc_sb[:, kk, mm, :],
                             gag[:, kk:kk + 1], start=(kk == 0),
                             stop=(kk == 3))
        stmp = ew.tile([128, 1], F32, tag="stmp")
        nc.scalar.activation(stmp[:], sp_ps[:], AF.Sigmoid,
                             bias=bsc_sb[:, mm:mm + 1])
        nc.scalar.activation(spectral[:, mm:mm + 1], stmp[:], AF.Sigmoid)

    # ---- MT = (Wr @ U1 diag(lam))^T, bf16 [4, 256] ----
    MT = consts.tile([4, 256], BF16)
    for mm in range(2):
        m_ps = ps_sm.tile([128, 4], F32, tag="sm")
        for kk in range(2):
            nc.tensor.matmul(m_ps[:], wrt_sb[:, kk, mm, :], U1n[:, kk, :],
                             start=(kk == 0), stop=(kk == 1))
        m_sb = ew.tile([128, 4], F32, tag="msb")
        nc.scalar.copy(m_sb[:], m_ps[:])
        mt_ps = ps_sm.tile([4, 128], F32, tag="sm")
        nc.tensor.transpose(mt_ps[:], m_sb[:], ident[:])
        nc.vector.tensor_scalar(MT[:, mm * 128:(mm + 1) * 128], mt_ps[:],
                                lam_sb[:], None, op0=ALU.mult)

    # ---- G[r, h, w] = U3T[r, h] * U2T[r, w], bf16 [4, 4096] ----
    G = consts.tile([4, HW], BF16)
    nc.vector.tensor_tensor(
        G[:].rearrange("p (h w) -> p h w", h=64),
        U3T[:, 0:64][:, :, None].broadcast_to([4, 64, 64]),
        U2T[:, 0:64][:, None, :].broadcast_to([4, 64, 64]),
        op=ALU.mult)

    # ---- final elementwise stage ----
    for pt in range(8):
        sp_ps = ps_spat.tile([128, 512], F32, tag="spat")
        nc.tensor.matmul(sp_ps[:], ones4[:], G[:, pt * 512:(pt + 1) * 512],
                         start=True, stop=True)
        sig_sp = ew.tile([128, 512], F32, tag="sig")
        nc.scalar.activation(sig_sp[:], sp_ps[:], AF.Sigmoid,
                             scale=float(ws), bias=float(bs))
        for ct in range(2):
            rc_ps = ps_rec.tile([128, 512], F32, tag="rec")
            nc.tensor.matmul(rc_ps[:], MT[:, ct * 128:(ct + 1) * 128],
                             G[:, pt * 512:(pt + 1) * 512], start=True,
                             stop=True)
            fmb = fmring.tile([128, 512], F32, tag="fmb")
            nc.sync.dma_start(fmb[:], fm_scr[ct, pt])
            frm_t = xrf[:, ct, 1 + pt * 8: 9 + pt * 8, 1:65]
            oth_t = xrf[:, 2 + ct, 1 + pt * 8: 9 + pt * 8, 1:65]
            spc = spectral[:, ct:ct + 1]
            t2 = ew.tile([128, 8, 64], F32, tag="t2")
            nc.scalar.activation(t2[:], oth_t, AF.Copy,
                                 scale=float(1.0 - alpha))
            A = ew.tile([128, 8, 64], F32, tag="A")
            nc.vector.scalar_tensor_tensor(A[:], frm_t, float(alpha), t2[:],
                                           op0=ALU.mult, op1=ALU.subtract)
            A2 = ew.tile([128, 8, 64], F32, tag="A2")
            nc.vector.scalar_tensor_tensor(
                A2[:], A[:], spc,
                sig_sp[:].rearrange("p (h w) -> p h w", h=8),
                op0=ALU.mult, op1=ALU.mult)
            fu_t = outr.tile([128, 8, 64], F32, tag="fu")
            nc.vector.tensor_tensor(fu_t[:], A2[:], t2[:], op=ALU.add)
            nc.sync.dma_start(
                fused_o[ct * 128:(ct + 1) * 128, pt * 8:(pt + 1) * 8, :],
                fu_t[:])
            r1 = ew.tile([128, 512], F32, tag="r1")
            nc.scalar.activation(r1[:], rc_ps[:], AF.Identity,
                                 bias=br_sb[:, ct:ct + 1])
            r2 = ew.tile([128, 512], F32, tag="r2")
            nc.vector.scalar_tensor_tensor(r2[:], r1[:], spc, sig_sp[:],
                                           op0=ALU.mult, op1=ALU.mult)
            cp_t = outr.tile([128, 512], F32, tag="cp")
            nc.vector.tensor_tensor(cp_t[:], r2[:], fmb[:], op=ALU.add)
            nc.sync.dma_start(
                cpr_o[ct * 128:(ct + 1) * 128, pt * 8:(pt + 1) * 8, :],
                cp_t[:].rearrange("p (h w) -> p h w", h=8))
    ctx.close()


def _prep_weights(W3, b3, Wa1, ba1, Wa2, ba2, Wa3, ba3, Wu, bu, Wr, br,
                  Wsa, bsa, Wsm, bsm):
    f = np.float32
    # w3t[p, kt, t, co] = W3[co, kt*128+p, dy, dx]
    w3t = np.ascontiguousarray(
        W3.reshape(C, 4, 128, 9).transpose(2, 1, 3, 0)).astype(f)
    b3h = np.ascontiguousarray(b3.reshape(2, 128).T).astype(f)
    # wa[0, s, m, n] = Wa_m[n, s]
    wa = np.ascontiguousarray(
        np.stack([Wa1, Wa2, Wa3], axis=0).transpose(2, 0, 1)[None]).astype(f)
    ba = np.ascontiguousarray(
        np.stack([ba1, ba2, ba3], axis=0).reshape(3, 2, 128)
        .transpose(2, 0, 1)).astype(f)
    wu = np.ascontiguousarray(
        Wu.reshape(K, 2, 128).transpose(2, 1, 0)).astype(f)
    buh = bu.reshape(4, 1).astype(f)
    # wrt[p, kk, mm, m] = Wr[mm*128+m, kk*128+p]
    wrt = np.ascontiguousarray(
        Wr.reshape(2, 128, 2, 128).transpose(3, 2, 0, 1)).astype(f)
    brh = np.ascontiguousarray(br.reshape(2, 128).T).astype(f)
    # wsc[p, kk, mm, m]: kk<2 -> Wsa/128 (mean folded), kk>=2 -> Wsm
    wsa_r = (Wsa / 128.0).reshape(2, 128, 2, 128).transpose(3, 2, 0, 1)
    wsm_r = Wsm.reshape(2, 128, 2, 128).transpose(3, 2, 0, 1)
    wsc = np.ascontiguousarray(
        np.concatenate([wsa_r, wsm_r], axis=1)).astype(f)
    bsc = np.ascontiguousarray((bsa + bsm).reshape(2, 128).T).astype(f)
    return dict(w3t=w3t, b3=b3h, wa=wa, ba=ba, wu=wu, bu=buh, wrt=wrt,
                br=brh, wsc=wsc, bsc=bsc)


_CACHE = {}


def kernel(frm_feat, other_feat, W3, b3, Wa1, ba1, Wa2, ba2, Wa3, ba3,
           Wu, bu, Wr, br, ws, bs, Wsa, bsa, Wsm, bsm, alpha, lam,
           _trace=False, _tmpdir=None):
    frm_feat = np.asarray(frm_feat, np.float32)
    other_feat = np.asarray(other_feat, np.float32)
    key = (float(alpha), float(ws), float(bs))
    if key not in _CACHE:
        _CACHE[key] = build_program(float(alpha), float(ws), float(bs))
    nc = _CACHE[key]

    wd = _prep_weights(np.asarray(W3), np.asarray(b3), np.asarray(Wa1),
                       np.asarray(ba1), np.asarray(Wa2), np.asarray(ba2),
                       np.asarray(Wa3), np.asarray(ba3), np.asarray(Wu),
                       np.asarray(bu), np.asarray(Wr), np.asarray(br),
                       np.asarray(Wsa), np.asarray(bsa), np.asarray(Wsm),
                       np.asarray(bsm))
    wd["lam"] = np.asarray(lam, np.float32).reshape(4, 1)

    in_maps = []
    for b_i in range(NCORES):
        m = dict(wd)
        m["frm"] = np.ascontiguousarray(frm_feat[b_i])
        m["oth"] = np.ascontiguousarray(other_feat[b_i])
        in_maps.append(m)

    res = bass_utils.run_bass_kernel_spmd(
        nc, in_maps, core_ids=list(range(NCORES)), trace=_trace,
        tmpdir=_tmpdir)
    fused = np.stack([res.results[i]["fused"] for i in range(NCORES)])
    cpr = np.stack([res.results[i]["cpr"] for i in range(NCORES)])
    kernel._last_exec_time_ns = res.exec_time_ns
    kernel._last_results = res
    return fused, cpr



# revision 15
# speedup vs baseline: 1.2415x; 1.2415x over previous
"""Trainium2 Bass kernel for nn_MDRMWithCPRecon (optimized v2).

Sharding: pure data parallel over batch B=8 -> one batch element per
NeuronCore. All parameters replicated.

Per-core pipeline:
  x  = cat(frm, oth)                    [512, 64, 64]
  Fm = lrelu(conv3x3(x, W3) + b3)      [256, 64, 64]   <- bulk of FLOPs
  U1/U2/U3 rank-4 softmax factors from pooled stats
  spatial  = sigmoid(ws * U3 @ U2^T + bs)
  spectral = sigmoid(sigmoid(Wsa@mean + Wsm@max + biases))
  Wt = spectral x spatial
  fused    = a*Wt*frm + (1-a)*(1-Wt)*oth
  cp_recon = (Wr @ cp + br) * Wt + Fm,  cp = rank-4 CP(U1,U2,U3,lam)

v2 changes vs baseline (294us):
  - whole conv in bf16 (tolerance 2e-2; measured err stays ~1e-3):
    halves input DMA and SBUF, same 1 cycle/row PE rate as f32r.
  - inputs DMA'd straight into the padded SBUF image (bitcast view),
    no staging tiles / tensor_copy casts; borders memset once.
  - DMA order interleaves per-kt weight and image chunks so the PE can
    start accumulating ~2us in and is fed at matched rate.
  - Fm stays in SBUF as bf16 (kills the 8MB fm_scratch DRAM roundtrip).
  - E' = ((1-a)/a)*oth - frm precomputed on Vector during the conv.
  - pooled-stats -> U1/U2/U3 softmax chain batched into one [4, 384]
    tile (one exp, one ones-matmul row-sum, one reciprocal, one
    broadcast) instead of ~90 tiny serialized ops.
  - final elementwise stage in bf16 (2x DVE), spread over
    Vector/Scalar/GpSimd; outputs stored bf16, cast to f32 on host.
"""

import numpy as np
import ml_dtypes

import concourse.bacc as bacc
import concourse.bass as bass
import concourse.tile as tile
from concourse import mybir, bass_utils

F32 = mybir.dt.float32
BF16 = mybir.dt.bfloat16
AF = mybir.ActivationFunctionType
ALU = mybir.AluOpType
AX = mybir.AxisListType

B, C, H, W, K = 8, 256, 64, 64, 4
HW = H * W
NCORES = 8
BF = ml_dtypes.bfloat16

# tap order: center tap first (full window for PSUM start=True)
TAPS = [(1, 1), (0, 0), (0, 1), (0, 2), (1, 0), (1, 2), (2, 0), (2, 1),
        (2, 2)]


def build_program(alpha, ws, bs):
    from concourse.masks import make_identity

    nc = bacc.Bacc("TRN2", target_bir_lowering=False, debug=False,
                   num_devices=NCORES)

    # ---- DRAM I/O (per core) ----
    frm_d = nc.dram_tensor("frm", [128, 2, H, W], BF16, kind="ExternalInput")
    oth_d = nc.dram_tensor("oth", [128, 2, H, W], BF16, kind="ExternalInput")
    w3t_d = nc.dram_tensor("w3t", [128, 4, 2, 9, 128], BF16,
                           kind="ExternalInput")
    b3_d = nc.dram_tensor("b3", [128, 2], F32, kind="ExternalInput")
    wa_d = nc.dram_tensor("wa", [1, 2, 3, 256], F32, kind="ExternalInput")
    wu_d = nc.dram_tensor("wu", [128, 2, 4], F32, kind="ExternalInput")
    cu_d = nc.dram_tensor("cu", [4, 384], F32, kind="ExternalInput")
    wrt_d = nc.dram_tensor("wrt", [128, 2, 256], F32, kind="ExternalInput")
    br_d = nc.dram_tensor("br", [128, 2], F32, kind="ExternalInput")
    wsc_d = nc.dram_tensor("wsc", [128, 4, 2, 128], F32, kind="ExternalInput")
    bsc_d = nc.dram_tensor("bsc", [128, 2], F32, kind="ExternalInput")
    lam_d = nc.dram_tensor("lam", [4, 1], F32, kind="ExternalInput")
    fused_o = nc.dram_tensor("fused", [128, 2, H, W], BF16,
                             kind="ExternalOutput")
    cpr_o = nc.dram_tensor("cpr", [128, 2, H, W], BF16,
                           kind="ExternalOutput")

    with tile.TileContext(nc) as tc:
        _build_tile(tc, nc, make_identity, locals(), alpha, ws, bs)
    nc.compile()
    return nc


def _build_tile(tc, nc, make_identity, T, alpha, ws, bs):
    frm_d, oth_d, w3t_d, b3_d = T["frm_d"], T["oth_d"], T["w3t_d"], T["b3_d"]
    wa_d, wu_d, cu_d, wrt_d = T["wa_d"], T["wu_d"], T["cu_d"], T["wrt_d"]
    br_d, wsc_d, bsc_d, lam_d = T["br_d"], T["wsc_d"], T["bsc_d"], T["lam_d"]
    fused_o, cpr_o = T["fused_o"], T["cpr_o"]

    import contextlib
    ctx = contextlib.ExitStack()
    consts = ctx.enter_context(tc.tile_pool(name="consts", bufs=1))
    ew = ctx.enter_context(tc.tile_pool(name="ew", bufs=3))
    outr = ctx.enter_context(tc.tile_pool(name="outr", bufs=3))
    ps_conv = ctx.enter_context(tc.tile_pool(name="ps_conv", bufs=2,
                                             space="PSUM"))
    ps_sm = ctx.enter_context(tc.tile_pool(name="ps_sm", bufs=2,
                                           space="PSUM"))
    ps_fin = ctx.enter_context(tc.tile_pool(name="ps_fin", bufs=2,
                                            space="PSUM"))

    # ---- persistent SBUF tiles ----
    xin = consts.tile([128, 4, 66, 66], BF16)     # padded cat(frm,oth)
    w3t = consts.tile([128, 4, 2, 9, 128], BF16)  # conv weights
    fm = consts.tile([128, 2, 8, 512], BF16)      # conv output Fm
    e_sb = consts.tile([128, 2, 8, 512], BF16)    # E' = k*oth - frm
    b3_sb = consts.tile([128, 2], F32)
    wa_sb = consts.tile([1, 2, 3, 256], F32)
    wu_sb = consts.tile([128, 2, 4], F32)
    cu_sb = consts.tile([4, 384], F32)
    wrt_sb = consts.tile([128, 2, 256], F32)
    br_sb = consts.tile([128, 2], F32)
    wsc_sb = consts.tile([128, 4, 2, 128], F32)
    bsc_sb = consts.tile([128, 2], F32)
    lam_sb = consts.tile([4, 1], F32)
    # stats
    sums1 = consts.tile([128, 2, 8], F32)      # per-(ct,pt) channel sums
    m1p = consts.tile([128, 2, 8], F32)        # per-(ct,pt) channel max
    pp_sum = consts.tile([128, 2, 8, 64], F32)  # per-pt sum over h -> [c,w]
    pp_max = consts.tile([128, 2, 8, 64], F32)
    pq_sum = consts.tile([128, 2, 2, 64], F32)  # [m2|m3][ct] pooled sums
    pq_max = consts.tile([128, 2, 2, 64], F32)
    stat1 = consts.tile([128, 4], F32)          # [sum ct0, sum ct1, max...]
    # U chain (pooled rows kept on partition 0 only)
    row1s = consts.tile([1, 256], F32)
    row1m = consts.tile([1, 256], F32)
    row23s = consts.tile([1, 128], F32)
    row23m = consts.tile([1, 128], F32)
    a_sb = consts.tile([128, 2, 384], F32)
    ub_sb = consts.tile([4, 384], F32)
    ue_sb = consts.tile([4, 384], F32)
    rec_sb = consts.tile([1, 384], F32)
    u_all = consts.tile([4, 384], F32)
    u1n = consts.tile([128, 2, 4], F32)
    gag = consts.tile([128, 4], F32)
    spec = consts.tile([128, 2], F32)
    spcA = consts.tile([128, 2], F32)
    mx2 = consts.tile([128, 128], F32)
    G = consts.tile([4, HW], BF16)    # spatial: U3[h]*U2[w]
    Gc = consts.tile([4, HW], BF16)   # cp recon: U2[h]*U3[w]
    MT = consts.tile([4, 256], BF16)

    ident = consts.tile([128, 128], F32)
    make_identity(nc, ident[:])
    ones128 = consts.tile([128, 1], F32)
    nc.gpsimd.memset(ones128[:], 1.0)
    ones41 = consts.tile([4, 1], F32)
    nc.gpsimd.memset(ones41[:], 1.0)
    ones14 = consts.tile([1, 4], F32)
    nc.gpsimd.memset(ones14[:], 1.0)
    ones4b = consts.tile([4, 128], BF16)
    nc.gpsimd.memset(ones4b[:], 1.0)

    # ---- zero the padded-image border ----
    nc.gpsimd.memset(xin[:, :, :, 0:1], 0.0)
    nc.gpsimd.memset(xin[:, :, :, 65:66], 0.0)
    nc.gpsimd.memset(xin[:, :, 0:1, :], 0.0)
    nc.gpsimd.memset(xin[:, :, 65:66, :], 0.0)

    # ---- DMA order: b3 first, then per-kt (weights ct0, image h0) pairs,
    # then ct1 weights, h1 image halves, then smalls ----
    nc.sync.dma_start(b3_sb[:], b3_d[:])
    srcs = [frm_d, frm_d, oth_d, oth_d]
    for kt in range(4):
        nc.sync.dma_start(w3t[:, kt, 0], w3t_d[:, kt, 0])
        nc.sync.dma_start(xin[:, kt, 1:33, 1:65],
                          srcs[kt][:, kt % 2, 0:32, :])
    for kt in range(4):
        nc.sync.dma_start(w3t[:, kt, 1], w3t_d[:, kt, 1])
    for kt in range(4):
        nc.sync.dma_start(xin[:, kt, 33:65, 1:65],
                          srcs[kt][:, kt % 2, 32:64, :])
    for sb, dd in ((wa_sb, wa_d), (wu_sb, wu_d), (cu_sb, cu_d),
                   (wrt_sb, wrt_d), (br_sb, br_d), (wsc_sb, wsc_d),
                   (bsc_sb, bsc_d), (lam_sb, lam_d)):
        nc.sync.dma_start(sb[:], dd[:])

    kk_e = float((1.0 - alpha) / alpha)

    # ---- conv3x3 + lrelu + streaming stats ----
    for pt in range(8):
        for ct in range(2):
            ps = ps_conv.tile([128, 8, 64], F32, tag="conv")
            idx = 0
            for kt in range(4):
                for (dy, dx) in TAPS:
                    nc.tensor.matmul(
                        ps[:],
                        w3t[:, kt, ct, dy * 3 + dx, :],
                        xin[:, kt, pt * 8 + dy: pt * 8 + dy + 8,
                            dx: dx + 64],
                        start=(idx == 0), stop=(idx == 35))
                    idx += 1
            nc.scalar.activation(fm[:, ct, pt].rearrange(
                "p (h w) -> p h w", h=8), ps[:], AF.Lrelu,
                bias=b3_sb[:, ct:ct + 1], alpha=0.01,
                accum_out=sums1[:, ct, pt:pt + 1])
            # E' precompute (no conv dependency; fills vector idle)
            nc.vector.scalar_tensor_tensor(
                e_sb[:, ct, pt].rearrange("p (h w) -> p h w", h=8),
                xin[:, 2 + ct, 1 + pt * 8: 9 + pt * 8, 1:65], kk_e,
                xin[:, ct, 1 + pt * 8: 9 + pt * 8, 1:65],
                op0=ALU.mult, op1=ALU.subtract)
            blk = fm[:, ct, pt].rearrange("p (h w) -> p h w", h=8)
            blk_t = fm[:, ct, pt].rearrange("p (h w) -> p w h", h=8)
            # mode3 (per-h) stats: disjoint slices, written directly
            nc.vector.tensor_reduce(pq_sum[:, 1, ct, pt * 8:(pt + 1) * 8],
                                    blk, axis=AX.X, op=ALU.add)
            nc.vector.tensor_reduce(pq_max[:, 1, ct, pt * 8:(pt + 1) * 8],
                                    blk, axis=AX.X, op=ALU.max)
            # mode2 (per-w) partials, combined after the loop
            nc.vector.tensor_reduce(pp_sum[:, ct, pt, :], blk_t,
                                    axis=AX.X, op=ALU.add)
            nc.vector.tensor_reduce(pp_max[:, ct, pt, :], blk_t,
                                    axis=AX.X, op=ALU.max)
            nc.vector.tensor_reduce(m1p[:, ct, pt:pt + 1], blk,
                                    axis=AX.XY, op=ALU.max)

    # ---- combine stats ----
    nc.vector.tensor_reduce(stat1[:, 0:2], sums1[:], axis=AX.X, op=ALU.add)
    nc.vector.tensor_reduce(stat1[:, 2:4], m1p[:], axis=AX.X, op=ALU.max)
    nc.vector.tensor_reduce(
        pq_sum[:, 0], pp_sum[:].rearrange("p c t w -> p c w t"),
        axis=AX.X, op=ALU.add)
    nc.vector.tensor_reduce(
        pq_max[:, 0], pp_max[:].rearrange("p c t w -> p c w t"),
        axis=AX.X, op=ALU.max)

    # mode1 rows via per-column transposes; avg-scales folded into wa
    rowdst = [(row1s, 0), (row1s, 128), (row1m, 0), (row1m, 128)]
    for j, (dst, off) in enumerate(rowdst):
        tpj = ps_sm.tile([1, 128], F32, tag="sm")
        nc.tensor.transpose(tpj[:], stat1[:, j:j + 1], ident[:])
        nc.scalar.copy(dst[0:1, off:off + 128], tpj[:])

    # mode2/3 sum rows: ones-matmul over channels, add ct halves
    srow = ps_sm.tile([1, 2, 2, 64], F32, tag="sm")
    nc.tensor.matmul(srow[:].rearrange("p a b c -> p (a b c)"), ones128[:],
                     pq_sum[:].rearrange("p a b c -> p (a b c)"),
                     start=True, stop=True)
    sr_sb = consts.tile([1, 2, 2, 64], F32)
    nc.scalar.copy(sr_sb[:], srow[:])
    nc.vector.tensor_tensor(row23s[0:1, :].rearrange("p (m w) -> p m w",
                                                     m=2),
                            sr_sb[:, :, 0, :], sr_sb[:, :, 1, :], op=ALU.add)
    # mode2/3 max rows: ct-combine, transpose, reduce, transpose back
    nc.vector.tensor_tensor(mx2[:].rearrange("p (m w) -> p m w", m=2),
                            pq_max[:, :, 0, :], pq_max[:, :, 1, :],
                            op=ALU.max)
    mxT = ps_sm.tile([128, 128], F32, tag="sm")
    nc.tensor.transpose(mxT[:], mx2[:], ident[:])
    mcol = ew.tile([128, 1], F32, tag="mcol")
    nc.vector.tensor_reduce(mcol[:], mxT[:], axis=AX.X, op=ALU.max)
    mrow = ps_sm.tile([1, 128], F32, tag="sm")
    nc.tensor.transpose(mrow[:], mcol[:], ident[:])
    nc.scalar.copy(row23m[:], mrow[:])

    # ---- a[o, n] = sum_s wa_s[o] * row_s[n]  (rank-1 outer products) ----
    for ct in range(2):
        ap_t = ps_sm.tile([128, 384], F32, tag="sm")
        cs = ct * 128
        nc.tensor.matmul(ap_t[:, 0:256], wa_sb[0:1, 0, 0, cs:cs + 128],
                         row1s[:], start=True, stop=False)
        nc.tensor.matmul(ap_t[:, 0:256], wa_sb[0:1, 1, 0, cs:cs + 128],
                         row1m[:], start=False, stop=True)
        nc.tensor.matmul(ap_t[:, 256:320], wa_sb[0:1, 0, 1, cs:cs + 128],
                         row23s[:, 0:64], start=True, stop=False)
        nc.tensor.matmul(ap_t[:, 256:320], wa_sb[0:1, 1, 1, cs:cs + 128],
                         row23m[:, 0:64], start=False, stop=True)
        nc.tensor.matmul(ap_t[:, 320:384], wa_sb[0:1, 0, 2, cs:cs + 128],
                         row23s[:, 64:128], start=True, stop=False)
        nc.tensor.matmul(ap_t[:, 320:384], wa_sb[0:1, 1, 2, cs:cs + 128],
                         row23m[:, 64:128], start=False, stop=True)
        nc.scalar.copy(a_sb[:, ct, :], ap_t[:])

    # ---- u = Wu @ a + (Wu@ba + bu)  [4, 384], then softmax over k ----
    u_ps = ps_sm.tile([4, 384], F32, tag="sm")
    nc.tensor.matmul(u_ps[:], wu_sb[:, 0, :], a_sb[:, 0, :], start=True,
                     stop=False)
    nc.tensor.matmul(u_ps[:], wu_sb[:, 1, :], a_sb[:, 1, :], start=False,
                     stop=True)
    nc.vector.tensor_tensor(ub_sb[:], u_ps[:], cu_sb[:], op=ALU.add)
    nc.scalar.activation(ue_sb[:], ub_sb[:], AF.Exp)
    ssum = ps_sm.tile([1, 384], F32, tag="sm")
    nc.tensor.matmul(ssum[:], ones41[:], ue_sb[:], start=True, stop=True)
    nc.vector.reciprocal(rec_sb[:], ssum[:])
    rb = ps_sm.tile([4, 384], F32, tag="sm")
    nc.tensor.matmul(rb[:], ones14[:], rec_sb[:], start=True, stop=True)
    nc.vector.tensor_tensor(u_all[:], ue_sb[:], rb[:], op=ALU.mult)

    # ---- G[r, h, w] = U3[h, r] * U2[w, r]  (bf16, spatial) ----
    nc.vector.tensor_tensor(
        G[:].rearrange("p (h w) -> p h w", h=64),
        u_all[:, 320:384][:, :, None].broadcast_to([4, 64, 64]),
        u_all[:, 256:320][:, None, :].broadcast_to([4, 64, 64]),
        op=ALU.mult)
    # ---- Gc[r, h, w] = U2[h, r] * U3[w, r]  (bf16, cp recon) ----
    nc.vector.tensor_tensor(
        Gc[:].rearrange("p (h w) -> p h w", h=64),
        u_all[:, 256:320][:, :, None].broadcast_to([4, 64, 64]),
        u_all[:, 320:384][:, None, :].broadcast_to([4, 64, 64]),
        op=ALU.mult)

    # ---- MT = (Wr @ U1 diag(lam))^T  [4, 256] bf16 ----
    for kk2 in range(2):
        u1t_ps = ps_sm.tile([128, 4], F32, tag="sm")
        nc.tensor.transpose(u1t_ps[:], u_all[0:4, kk2 * 128:(kk2 + 1) * 128],
                            ident[0:4, 0:4])
        nc.scalar.copy(u1n[:, kk2, :], u1t_ps[:])
    mt_ps = ps_sm.tile([4, 256], F32, tag="sm")
    nc.tensor.matmul(mt_ps[:], u1n[:, 0, :], wrt_sb[:, 0, :], start=True,
                     stop=False)
    nc.tensor.matmul(mt_ps[:], u1n[:, 1, :], wrt_sb[:, 1, :], start=False,
                     stop=True)
    nc.vector.tensor_scalar(MT[:], mt_ps[:], lam_sb[:], None, op0=ALU.mult)

    # ---- spectral attention ----
    f_ps = ps_sm.tile([128, 2, 128], F32, tag="sm")
    for ct in range(2):
        nc.tensor.matmul(f_ps[:, ct, :],
                         u_all[0:4, ct * 128:(ct + 1) * 128],
                         u_all[:, 256:384], start=True, stop=True)
        nc.vector.tensor_reduce(gag[:, ct:ct + 1], f_ps[:, ct, :],
                                axis=AX.X, op=ALU.add)
        nc.vector.tensor_reduce(gag[:, 2 + ct:3 + ct], f_ps[:, ct, :],
                                axis=AX.X, op=ALU.max)
    spv = ps_sm.tile([128, 2], F32, tag="sm")
    for mm in range(2):
        for kk2 in range(4):
            nc.tensor.matmul(spv[:, mm:mm + 1], wsc_sb[:, kk2, mm, :],
                             gag[:, kk2:kk2 + 1], start=(kk2 == 0),
                             stop=(kk2 == 3))
    for mm in range(2):
        stmp = ew.tile([128, 1], F32, tag="stmp")
        nc.scalar.activation(stmp[:], spv[:, mm:mm + 1], AF.Sigmoid,
                             bias=bsc_sb[:, mm:mm + 1])
        nc.scalar.activation(spec[:, mm:mm + 1], stmp[:], AF.Sigmoid)
    nc.vector.tensor_scalar(spcA[:], spec[:], float(-alpha), None,
                            op0=ALU.mult)

    # ---- final stage ----
    for pt in range(8):
        sp_ps = ps_fin.tile([128, 8, 64], F32, tag="spat")
        nc.tensor.matmul(sp_ps[:].rearrange("p a b -> p (a b)"), ones4b[:],
                         G[:, pt * 512:(pt + 1) * 512], start=True,
                         stop=True)
        s_t = ew.tile([128, 8, 64], BF16, tag="sig")
        nc.scalar.activation(s_t[:], sp_ps[:], AF.Sigmoid, scale=float(ws),
                             bias=float(bs))
        for ct in range(2):
            rc_ps = ps_fin.tile([128, 8, 64], F32, tag="rec")
            nc.tensor.matmul(rc_ps[:].rearrange("p a b -> p (a b)"),
                             MT[:, ct * 128:(ct + 1) * 128],
                             Gc[:, pt * 512:(pt + 1) * 512], start=True,
                             stop=True)
            r1 = ew.tile([128, 8, 64], BF16, tag="r1")
            nc.scalar.activation(r1[:], rc_ps[:], AF.Identity,
                                 bias=br_sb[:, ct:ct + 1])
            A2 = ew.tile([128, 8, 64], BF16, tag="A2")
            nc.vector.scalar_tensor_tensor(
                A2[:], e_sb[:, ct, pt].rearrange("p (h w) -> p h w", h=8),
                spcA[:, ct:ct + 1], s_t[:], op0=ALU.mult, op1=ALU.mult)
            fu = outr.tile([128, 8, 64], BF16, tag="fu")
            nc.vector.scalar_tensor_tensor(
                fu[:], xin[:, 2 + ct, 1 + pt * 8: 9 + pt * 8, 1:65],
                float(1.0 - alpha), A2[:], op0=ALU.mult, op1=ALU.add)
            nc.sync.dma_start(fused_o[:, ct, pt * 8:(pt + 1) * 8, :], fu[:])
            r2 = ew.tile([128, 8, 64], BF16, tag="r2")
            nc.vector.scalar_tensor_tensor(
                r2[:], r1[:], spec[:, ct:ct + 1], s_t[:], op0=ALU.mult,
                op1=ALU.mult)
            cp = outr.tile([128, 8, 64], BF16, tag="cp")
            nc.vector.tensor_tensor(
                cp[:], r2[:],
                fm[:, ct, pt].rearrange("p (h w) -> p h w", h=8),
                op=ALU.add)
            nc.sync.dma_start(cpr_o[:, ct, pt * 8:(pt + 1) * 8, :], cp[:])
    ctx.close()


def _prep_weights(W3, b3, Wa1, ba1, Wa2, ba2, Wa3, ba3, Wu, bu, Wr, br,
                  Wsa, bsa, Wsm, bsm):
    f = np.float32
    # w3t[p, kt, ct, t, co] = W3[ct*128+co, kt*128+p, dy, dx]
    w3t = np.ascontiguousarray(
        W3.reshape(2, 128, 4, 128, 9).transpose(3, 2, 0, 4, 1)).astype(BF)
    b3h = np.ascontiguousarray(b3.reshape(2, 128).T).astype(f)
    # wa[0, s, m, o] = Wa_m[o, s]; avg column scaled by 1/pool_n
    was = []
    for m, wv in enumerate((Wa1, Wa2, Wa3)):
        wv = np.array(wv, f).copy()
        wv[:, 0] /= (HW if m == 0 else C * H)
        was.append(wv)
    wa = np.ascontiguousarray(
        np.stack(was, axis=0).transpose(2, 0, 1)[None]).astype(f)
    # wu[p, ct, k] = Wu[k, ct*128+p]
    wu = np.ascontiguousarray(
        Wu.reshape(K, 2, 128).transpose(2, 1, 0)).astype(f)
    # cu[k, n] = (Wu @ ba_m + bu)[k] for n in mode-m block
    cus = [Wu @ bam + bu for bam in (ba1, ba2, ba3)]
    cu = np.concatenate([np.tile(cus[0][:, None], (1, 256)),
                         np.tile(cus[1][:, None], (1, 64)),
                         np.tile(cus[2][:, None], (1, 64))], axis=1)
    cu = np.ascontiguousarray(cu).astype(f)
    # wrt[p, kk, m] = Wr[m, kk*128+p]
    wrt = np.ascontiguousarray(
        Wr.reshape(256, 2, 128).transpose(2, 1, 0)).astype(f)
    brh = np.ascontiguousarray(br.reshape(2, 128).T).astype(f)
    # wsc[p, kk, mm, m]: kk<2 -> Wsa/(W+H) (mean folded), kk>=2 -> Wsm
    wsa_r = (Wsa / 128.0).reshape(2, 128, 2, 128).transpose(3, 2, 0, 1)
    wsm_r = Wsm.reshape(2, 128, 2, 128).transpose(3, 2, 0, 1)
    wsc = np.ascontiguousarray(
        np.concatenate([wsa_r, wsm_r], axis=1)).astype(f)
    bsc = np.ascontiguousarray((bsa + bsm).reshape(2, 128).T).astype(f)
    return dict(w3t=w3t, b3=b3h, wa=wa, wu=wu, cu=cu, wrt=wrt, br=brh,
                wsc=wsc, bsc=bsc)


_CACHE = {}


def kernel(frm_feat, other_feat, W3, b3, Wa1, ba1, Wa2, ba2, Wa3, ba3,
           Wu, bu, Wr, br, ws, bs, Wsa, bsa, Wsm, bsm, alpha, lam,
           _trace=False, _tmpdir=None):
    key = (float(alpha), float(ws), float(bs))
    if key not in _CACHE:
        _CACHE[key] = build_program(float(alpha), float(ws), float(bs))
    nc = _CACHE[key]

    wd = _prep_weights(np.asarray(W3, np.float32), np.asarray(b3),
                       np.asarray(Wa1), np.asarray(ba1), np.asarray(Wa2),
                       np.asarray(ba2), np.asarray(Wa3), np.asarray(ba3),
                       np.asarray(Wu), np.asarray(bu),
                       np.asarray(Wr, np.float32), np.asarray(br),
                       np.asarray(Wsa, np.float32), np.asarray(bsa),
                       np.asarray(Wsm, np.float32), np.asarray(bsm))
    wd["lam"] = np.asarray(lam, np.float32).reshape(4, 1)

    in_maps = []
    for b_i in range(NCORES):
        m = dict(wd)
        m["frm"] = frm_bat(frm_feat, b_i)
        m["oth"] = frm_bat(other_feat, b_i)
        in_maps.append(m)

    res = bass_utils.run_bass_kernel_spmd(
        nc, in_maps, core_ids=list(range(NCORES)), trace=_trace,
        tmpdir=_tmpdir)
    fused = np.stack([_unshard(res.results[i]["fused"])
                      for i in range(NCORES)])
    cpr = np.stack([_unshard(res.results[i]["cpr"])
                    for i in range(NCORES)])
    kernel._last_exec_time_ns = res.exec_time_ns
    kernel._last_results = res
    return fused, cpr


def frm_bat(x, b_i):
    """[B, 256, H, W] f32 -> [128, 2, H, W] bf16 for batch b_i."""
    return np.ascontiguousarray(
        np.asarray(x[b_i], np.float32).reshape(2, 128, H, W)
        .transpose(1, 0, 2, 3)).astype(BF)


def _unshard(a):
    """[128, 2, H, W] bf16 -> [256, H, W] f32."""
    return np.ascontiguousarray(
        np.asarray(a).transpose(1, 0, 2, 3)).reshape(256, H, W)\
        .astype(np.float32)
